# revision 21
# baseline (speedup 1.0000x reference)
"""Trainium2 Bass kernel for nn_AttentionHead (B=2, T=2048, C=2048, H=16 heads, D=128).

Sharding: tensor-parallel over heads — 2 heads per NeuronCore (8 cores).
Each core computes qkv for its heads, RoPE, causal softmax attention, and a
partial c_proj contribution; the host sums the 8 partial outputs.

v2 layout strategy (per core, heads hA=2c, hB=2c+1):
  - All matmul operands are bf16 (psum accumulation stays f32): halves DMA
    and SBUF traffic, enables fast weight load, same 1 cyc/row PE rate.
  - x is pre-transposed on the host to xT [C, B*T] bf16; one coalesced DMA
    per 512-token block loads all 16 contraction tiles.
  - q/k weight rows are permuted so RoPE pairs (2i, 2i+1) become (top i, bot
    64+i) and the qkv matmul emits [Qtop|Ktop] / [Qbot|Kbot] psum tiles whose
    partitions are lane-aligned for the RoPE vector ops (f32 math, bf16 out).
  - Scores are computed transposed (S.T[k, q]); exp -> bf16 stripes.
  - Softmax denominator via an all-ones 128x128 stationary matmul, which
    replicates the k-sum across all 128 psum partitions; a single fast
    approximate reciprocal + one DVE multiply normalizes the PV output
    (no single-partition reciprocal, no broadcast matmul).
  - c_proj partials evicted as bf16 (ACT/DVE alternating) and written with
    one DMA per 128-token row tile on the scalar HWDGE queue so output
    stores never block input loads on the sync queue.
"""

import sys

sys.path.insert(0, "/opt/trn_rl_repo")

import numpy as np
import ml_dtypes

import concourse.bass as bass
import concourse.mybir as mybir
import concourse.tile as tile
from concourse import bacc
from concourse.bass_utils import run_bass_kernel_spmd

F32 = mybir.dt.float32
BF16 = mybir.dt.bfloat16

B, T, C, H, D = 2, 2048, 2048, 16, 128
NC_CORES = 8
HPC = H // NC_CORES            # heads per core = 2
BT = B * T                     # 4096
NKT = C // 128                 # 16 contraction tiles
TBS = 512                      # token block size
NTB_B = T // TBS               # 4 token blocks per batch
INV_SQRT_D = 1.0 / float(np.sqrt(D))

_CACHE = {}
_CFG = {"B": B, "T": T, "C": C}


def _build_program():
    B, T, C = _CFG["B"], _CFG["T"], _CFG["C"]
    NKT = C // 128
    NTB_B = T // TBS
    BT = B * T
    nc = bacc.Bacc(None)

    xT = nc.dram_tensor("xT", [C, BT], BF16, kind="ExternalInput")
    wqk = nc.dram_tensor("wqk", [128, NKT, 4 * 128], BF16, kind="ExternalInput")
    wv = nc.dram_tensor("wv", [128, NKT, HPC * D], BF16, kind="ExternalInput")
    wp = nc.dram_tensor("wp", [128, HPC, C], BF16, kind="ExternalInput")
    cs = nc.dram_tensor("cs", [128, T], F32, kind="ExternalInput")
    sn = nc.dram_tensor("sn", [128, T], F32, kind="ExternalInput")
    ones_mat = nc.dram_tensor("ones_mat", [128, 128], BF16, kind="ExternalInput")
    out_d = nc.dram_tensor("out", [BT, C], BF16, kind="ExternalOutput")

    xr = xT[:, :].rearrange("(ko ki) t -> ki ko t", ki=128)

    with tile.TileContext(nc) as tc:
        with (
            tc.tile_pool(name="const", bufs=1) as constp,
            tc.tile_pool(name="xp", bufs=2) as xp,
            tc.tile_pool(name="qk", bufs=1) as qkp,
            tc.tile_pool(name="vp", bufs=1) as vp,
            tc.tile_pool(name="yp", bufs=1) as yp,
            tc.tile_pool(name="pp", bufs=2) as pp,
            tc.tile_pool(name="tmp", bufs=4) as tmpp,
            tc.tile_pool(name="rt", bufs=2) as rtp_pool,
            tc.tile_pool(name="rc", bufs=2) as rcp,
            tc.tile_pool(name="outp", bufs=3) as outp,
            tc.tile_pool(name="psA", bufs=3, space="PSUM") as psA,
            tc.tile_pool(name="psB", bufs=2, space="PSUM") as psB,
        ):
            # Loads are ordered so the first qk matmul only waits for the
            # first wqk chunk + first x chunk; everything else arrives behind
            # compute.  Stage-only constants are issued right before the
            # first stage that needs them.
            wqk_s = constp.tile([128, NKT, 4 * 128], BF16, tag="wqk")
            nc.sync.dma_start(wqk_s[:, 0:2, :], wqk[:, 0:2, :])
            nc.sync.dma_start(wqk_s[:, 2:4, :], wqk[:, 2:4, :])
            wv_s = constp.tile([128, NKT, HPC * D], BF16, tag="wv")
            wp_s = constp.tile([128, HPC, C], BF16, tag="wp")
            ones_s = constp.tile([128, 128], BF16, tag="ones")
            cs_s = constp.tile([128, T], F32, tag="cs")
            sn_s = constp.tile([128, T], F32, tag="sn")

            for b in range(B):
                # ---------------- stage A: qkv + rope -----------------------
                QH = qkp.tile([128, HPC, T], BF16, tag="QH")
                KH = qkp.tile([128, HPC, T], BF16, tag="KH")
                VH = vp.tile([128, NKT, HPC * D], BF16, tag="VH")
                yT = yp.tile([128, HPC, T], BF16, tag="yT")
                for tbl in range(NTB_B):
                    t0 = b * T + tbl * TBS
                    xt = xp.tile([128, NKT, TBS], BF16, tag="x")
                    if b == 0 and tbl == 0:
                        xchunks = [(0, 2), (2, 4), (4, 8), (8, 12), (12, 16)]
                    else:
                        xchunks = [(0, 4), (4, 8), (8, 12), (12, 16)]
                    for lo, hi in xchunks:
                        nc.sync.dma_start(
                            xt[:, lo:hi, :], xr[:, lo:hi, t0 : t0 + TBS]
                        )
                    if b == 0 and tbl == 0:
                        for wc in range(1, 4):
                            nc.sync.dma_start(
                                wqk_s[:, 4 * wc : 4 * wc + 4, :],
                                wqk[:, 4 * wc : 4 * wc + 4, :],
                            )
                        nc.sync.dma_start(wv_s, wv[:, :, :])
                        nc.sync.dma_start(cs_s, cs[:, :])
                        nc.sync.dma_start(sn_s, sn[:, :])
                    qkT = psA.tile([128, 1024], F32, tag="A")  # [Qtop | Ktop]
                    qkB = psA.tile([128, 1024], F32, tag="A")  # [Qbot | Kbot]
                    vps0 = psB.tile([128, 512], F32, tag="B")  # toks 0-255
                    vps1 = psB.tile([128, 512], F32, tag="B")  # toks 256-511
                    vtiles = (vps0, vps0, vps1, vps1)
                    for k in range(NKT):
                        st, sp = (k == 0), (k == NKT - 1)
                        nc.tensor.matmul(qkT[:, 0:512], wqk_s[:, k, 0:128], xt[:, k, :], start=st, stop=sp)
                        nc.tensor.matmul(qkB[:, 0:512], wqk_s[:, k, 128:256], xt[:, k, :], start=st, stop=sp)
                        nc.tensor.matmul(qkT[:, 512:1024], wqk_s[:, k, 256:384], xt[:, k, :], start=st, stop=sp)
                        nc.tensor.matmul(qkB[:, 512:1024], wqk_s[:, k, 384:512], xt[:, k, :], start=st, stop=sp)
                    # v matmuls run after qk so rope (which only needs qkT/qkB)
                    # overlaps them, and so the first-block wv load is hidden
                    # behind the qk matmul stream.
                    for k in range(NKT):
                        st, sp = (k == 0), (k == NKT - 1)
                        for s in range(4):
                            nc.tensor.matmul(
                                vtiles[s][:, (s % 2) * 256 : (s % 2) * 256 + 256],
                                xt[:, k, s * 128 : (s + 1) * 128],
                                wv_s[:, k, :],
                                start=(st and s % 2 == 0),
                                stop=(sp and s % 2 == 1),
                            )
                    # rope over [Q|K] jointly: cos/sin broadcast to 1024 wide
                    tcols = slice(tbl * TBS, (tbl + 1) * TBS)
                    c_b = cs_s[:, None, tcols].broadcast_to([128, 2, TBS])
                    s_b = sn_s[:, None, tcols].broadcast_to([128, 2, TBS])
                    t1 = tmpp.tile([128, 2, TBS], F32, tag="t")
                    nc.vector.tensor_mul(t1, qkT.rearrange("p (a n) -> p a n", a=2), c_b)
                    t3 = tmpp.tile([128, 2, TBS], F32, tag="t")
                    nc.vector.tensor_mul(t3, qkT.rearrange("p (a n) -> p a n", a=2), s_b)
                    t2 = tmpp.tile([128, 2, TBS], F32, tag="t")
                    nc.vector.tensor_mul(t2, qkB.rearrange("p (a n) -> p a n", a=2), s_b)
                    rtop = rtp_pool.tile([128, 2, TBS], BF16, tag="rt")
                    nc.vector.tensor_sub(rtop, t1, t2)
                    t4 = tmpp.tile([128, 2, TBS], F32, tag="t")
                    nc.vector.tensor_mul(t4, qkB.rearrange("p (a n) -> p a n", a=2), c_b)
                    rbot = rtp_pool.tile([128, 2, TBS], BF16, tag="rt")
                    nc.vector.tensor_add(rbot, t3, t4)
                    # regather halves into per-head layout (cross-partition -> DMA)
                    for h in range(HPC):
                        hs = slice(h * 64, (h + 1) * 64)
                        nc.sync.dma_start(QH[0:64, h, tcols], rtop[hs, 0, :])
                        nc.sync.dma_start(QH[64:128, h, tcols], rbot[hs, 0, :])
                        nc.sync.dma_start(KH[0:64, h, tcols], rtop[hs, 1, :])
                        nc.sync.dma_start(KH[64:128, h, tcols], rbot[hs, 1, :])
                    # v eviction psum -> sbuf (ACT copy, cast to bf16)
                    for s in range(4):
                        nc.scalar.activation(
                            VH[:, tbl * 4 + s, :],
                            vtiles[s][:, (s % 2) * 256 : (s % 2) * 256 + 256],
                            mybir.ActivationFunctionType.Copy,
                        )

                # ------- stage B + C: attention, c_proj interleaved ---------
                # j-outer so each 512-token q block's c_proj tiles run right
                # after both heads finish it: output DMA spreads over the
                # whole attention phase instead of piling up at the end.
                if b == 0:
                    nc.sync.dma_start(ones_s, ones_mat[:, :])
                    nc.sync.dma_start(wp_s, wp[:, :, :])

                def cproj_tiles(i_lo, i_hi):
                    # c_proj row tiles; evictions split DVE/GPSIMD in 512-col
                    # quarters so ACT stays free for exp and no single engine
                    # paces the eviction.
                    for i in range(i_lo, i_hi):
                        row0 = b * T + i * 128
                        ot = outp.tile([128, C], BF16, tag="o")
                        for n2 in range(C // 1024):
                            ps = psA.tile([128, 1024], F32, tag="A")
                            for n in range(2):
                                col = (n2 * 2 + n) * 512
                                for hh in range(HPC):
                                    nc.tensor.matmul(
                                        ps[:, n * 512 : (n + 1) * 512],
                                        yT[:, hh, i * 128 : (i + 1) * 128],
                                        wp_s[:, hh, col : col + 512],
                                        start=(hh == 0),
                                        stop=(hh == HPC - 1),
                                    )
                            oc = n2 * 1024
                            nc.vector.tensor_copy(ot[:, oc : oc + 512], ps[:, 0:512])
                            nc.scalar.activation(
                                ot[:, oc + 512 : oc + 1024],
                                ps[:, 512:1024],
                                mybir.ActivationFunctionType.Copy,
                            )
                            nc.scalar.dma_start(
                                out_d[row0 : row0 + 128, oc : oc + 1024],
                                ot[:, oc : oc + 1024],
                            )

                for j in range(NTB_B):
                    for h in range(HPC):
                        n_k = 4 * (j + 1)
                        n_full = 4 * j
                        qsl = slice(j * TBS, (j + 1) * TBS)
                        # diagonal chunks only compute the causal q range:
                        # chunk 4j+r covers q cols [128r, 512) of this block.
                        offs, wids = [], []
                        off = 0
                        for m in range(n_k):
                            r = m - n_full
                            w = TBS if r < 0 else TBS - 128 * r
                            offs.append(off)
                            wids.append(w)
                            off += w
                        p_stripe = pp.tile([128, 16 * 512], BF16, tag="P")

                        groups = [[m, m + 1] for m in range(0, n_full, 2)]
                        groups += [[n_full, n_full + 1], [n_full + 2, n_full + 3]]
                        for grp in groups:
                            gw = sum(wids[mm] for mm in grp)
                            g0 = offs[grp[0]]
                            sg = psA.tile([128, 1024], F32, tag="A")
                            sgoff = 0
                            for mm in grp:
                                r = mm - n_full
                                q0 = 0 if r < 0 else 128 * r
                                nc.tensor.matmul(
                                    sg[:, sgoff : sgoff + wids[mm]],
                                    KH[:, h, mm * 128 : (mm + 1) * 128],
                                    QH[:, h, j * TBS + q0 : (j + 1) * TBS],
                                    start=True,
                                    stop=True,
                                )
                                sgoff += wids[mm]
                            nc.scalar.activation(
                                p_stripe[:, g0 : g0 + gw],
                                sg[:, 0:gw],
                                mybir.ActivationFunctionType.Exp,
                                scale=INV_SQRT_D,
                            )
                            # causal mask on diagonal chunks as soon as exp lands
                            for mm in grp:
                                r = mm - n_full
                                if r >= 0:
                                    ck = p_stripe[:, offs[mm] : offs[mm] + wids[mm]]
                                    nc.gpsimd.affine_select(
                                        out=ck,
                                        in_=ck,
                                        compare_op=mybir.AluOpType.is_ge,
                                        fill=0.0,
                                        base=0,
                                        pattern=[[1, wids[mm]]],
                                        channel_multiplier=-1,
                                    )
                        den = psB.tile([128, 512], F32, tag="B")
                        pv = psB.tile([128, 512], F32, tag="B")
                        for m in range(n_k):
                            r = m - n_full
                            q0 = 0 if r < 0 else 128 * r
                            pck = p_stripe[:, offs[m] : offs[m] + wids[m]]
                            nc.tensor.matmul(
                                den[:, q0:512], ones_s, pck,
                                start=(m == 0), stop=(m == n_k - 1),
                            )
                            nc.tensor.matmul(
                                pv[:, q0:512], VH[:, m, h * D : (h + 1) * D], pck,
                                start=(m == 0), stop=(m == n_k - 1),
                            )
                        rden = rcp.tile([128, 512], F32, tag="rc")
                        nc.vector.reciprocal_approx_fast(out=rden, in_=den)
                        nc.vector.tensor_mul(yT[:, h, qsl], pv, rden)

                    # c_proj lags one q block so its matmuls never wait on the
                    # recip+mul chain of the block just finished.
                    if j > 0:
                        cproj_tiles(4 * (j - 1), 4 * j)
                cproj_tiles(4 * (NTB_B - 1), 4 * NTB_B)

    nc.compile()
    return nc


def _host_prep(x, w_atten, w_proj):
    """Build the shared + per-core input arrays."""
    B, T, C = _CFG["B"], _CFG["T"], _CFG["C"]
    NKT = C // 128
    BT = B * T
    x = np.asarray(x, dtype=np.float32)
    w_atten = np.asarray(w_atten, dtype=np.float32)
    w_proj = np.asarray(w_proj, dtype=np.float32)

    xT = np.ascontiguousarray(x.reshape(BT, C).T.astype(ml_dtypes.bfloat16))

    wq = w_atten[0:C]
    wk = w_atten[C : 2 * C]
    wv_full = w_atten[2 * C : 3 * C]

    # rope tables: theta_i = base^(-2i/D)
    theta = 1.0 / (10000.0 ** (np.arange(0, D, 2, dtype=np.float64) / D))  # [64]
    tpos = np.arange(T, dtype=np.float64)
    ang = np.outer(theta, tpos)  # [64, T]
    cs_half = np.cos(ang).astype(np.float32)
    sn_half = np.sin(ang).astype(np.float32)
    cs = np.ascontiguousarray(np.concatenate([cs_half, cs_half], axis=0))  # [128, T]
    sn = np.ascontiguousarray(np.concatenate([sn_half, sn_half], axis=0))

    ones_mat = np.ones((128, 128), dtype=ml_dtypes.bfloat16)

    top_idx = np.arange(0, D, 2)   # 64
    bot_idx = np.arange(1, D, 2)

    in_maps = []
    for c in range(NC_CORES):
        heads = [HPC * c + h for h in range(HPC)]
        # fb0 (tops of q), fb1 (bots of q), fb2/fb3 same for k
        fb = []
        for wmat in (wq, wk):
            for idx in (top_idx, bot_idx):
                rows = np.concatenate([wmat[hh * D + idx] for hh in heads], axis=0)
                fb.append(rows)  # [128, C]
        w_qk_c = np.concatenate(fb, axis=0)  # [512, C]
        wqk_dev = np.ascontiguousarray(
            w_qk_c.T.reshape(NKT, 128, 4 * 128).transpose(1, 0, 2)
        ).astype(ml_dtypes.bfloat16)
        w_v_c = np.concatenate([wv_full[hh * D : (hh + 1) * D] for hh in heads], axis=0)
        wv_dev = np.ascontiguousarray(
            w_v_c.T.reshape(NKT, 128, HPC * D).transpose(1, 0, 2)
        ).astype(ml_dtypes.bfloat16)
        cols = np.concatenate([np.arange(hh * D, (hh + 1) * D) for hh in heads])
        w_p_c = np.ascontiguousarray(w_proj[:, cols].T)  # [256, C]
        wp_dev = np.ascontiguousarray(
            w_p_c.reshape(HPC, 128, C).transpose(1, 0, 2)
        ).astype(ml_dtypes.bfloat16)
        in_maps.append(
            {
                "xT": xT,
                "wqk": wqk_dev,
                "wv": wv_dev,
                "wp": wp_dev,
                "cs": cs,
                "sn": sn,
                "ones_mat": ones_mat,
            }
        )
    return in_maps


def _execute(in_maps, trace=False, trace_kwargs=None):
    if "nc" not in _CACHE:
        _CACHE["nc"] = _build_program()
    nc = _CACHE["nc"]
    kwargs = {}
    if trace:
        _install_ntff_hook()
        kwargs["trace"] = True
        if trace_kwargs:
            kwargs.update(trace_kwargs)
    return run_bass_kernel_spmd(nc, in_maps, core_ids=list(range(NC_CORES)), **kwargs)


def _install_ntff_hook():
    """Restore the axon NTFF profile hook (the container's antenv lacks it)."""
    import types

    if "antenv.axon_hooks" in sys.modules:
        return
    mod = types.ModuleType("antenv.axon_hooks")
    mod._hook = None

    def set_axon_ntff_profile_hook(h):
        mod._hook = h

    def get_axon_ntff_profile_hook():
        if mod._hook is None:
            try:
                from trn_agent_boot.trn_boot import _ntff_profile_via_ctypes

                mod._hook = _ntff_profile_via_ctypes("/opt/axon/libaxon_pjrt.so")
            except Exception:
                mod._hook = None
        return mod._hook

    mod.set_axon_ntff_profile_hook = set_axon_ntff_profile_hook
    mod.get_axon_ntff_profile_hook = get_axon_ntff_profile_hook
    sys.modules["antenv.axon_hooks"] = mod


def kernel(x, w_atten, w_proj):
    in_maps = _host_prep(x, w_atten, w_proj)
    res = _execute(in_maps)
    total = res.results[0]["out"].astype(np.float32)
    for c in range(1, NC_CORES):
        total = total + res.results[c]["out"].astype(np.float32)
    return total.reshape(B, T, C)
